# revision 7
# baseline (speedup 1.0000x reference)
"""Trainium2 Bass kernel v2: fused residual-add + RMSNorm + local
(sliding-window) attention + output projection, sharded over 8 NeuronCores.

Sharding: 8 cores = (batch 4) x (sequence halves 2). Each core owns 2048
tokens of one batch row plus a 64-token halo of keys/values.

v2 changes vs baseline (TimelineSim per-core: 463us -> 320us; the
Tensor engine runs at the exact fp16 row-streaming roofline, 267us
busy = 84% occupancy; qTz zero-halves live in two manually-alternated
singles buffers zeroed once like weights; block 0's pad/halo tile
skips its never-read q projection):
- fp16 input/output DMA (host up/downcasts); halves HBM traffic. rel_err
  stays ~9e-4 against the fp32 reference (tolerance 2e-2).
- All matmuls stay fp16: fp8 (even QK-only) was measured at ~5e-2 output
  error - quantization noise in weighted sums scales with the signal, so
  softmax averaging does NOT crush it. DoubleRow fp8 paths remain behind
  the qk_fp8 flag for reference but default OFF.
- RMSNorm rsqrt via DVE Newton iteration on a bit-trick seed (int32
  tensor_scalar ops): the Activation engine never touches the Ln/Sqrt
  tables, so the Exp table loads exactly once per run (was 30 loads at
  1.28us each, alternating natural_log and exp_and_others).
- RMS second moment via Activation Square+accum_out (tensor_tensor_reduce
  crashes real HW with NRT_EXEC_UNIT_UNRECOVERABLE; bn_stats works but
  costs more DVE time).
- Masks are multiplicative fp16 applied after exp (exp(s+m) = exp(s)*M
  since M is constant per band), moving mask work off the f32 PSUM path
  and shrinking the mask tile to [128,2,64] via a stride-0 broadcast.
- Paired-head score matmuls: one matmul per head-pair via a 3D moving AP
  over an interleaved qTz layout. The zero half-partitions are required:
  K=64 matmuls with tile_position row offsets crash the device (the
  toolchain's known K<128 issue). Zeros are built with a 4x-mode
  broadcast tensor_copy, not memset (memset has no DVE fast modes).
- Attention software pipeline: per out-tile, scores/exp/mask/PV for both
  head-banks, then the DVE normalizes; the PE-heavy yT transpose + output
  projection runs one tile behind, and the out DMA one stage behind that,
  so neither the Tensor engine nor the Act queue ever waits on the
  normalize chain. Transposes share the projection PSUM pool so y gets
  double-buffered within the 8 PSUM banks.
- DMA queues: inputs + res on SP, out on Act - an output DMA's semaphore
  wait never blocks input prefetch. Weight DMAs are split per k-tile and
  emitted after block 0's input loads: the norm pipeline starts while the
  8MB of weights streams in, cutting a 37us dead PE gap at startup to
  ~15us. GPSIMD is never used for data ops (each op costs ~7ms wall on
  this axon path). DMA-transpose (XBAR) was tried for xT/yT and is slower
  end-to-end than PE transposes + batched DVE copies.
"""

import sys

for _p in ("/opt/trn_rl_repo", "/opt/pypackages"):
    if _p not in sys.path:
        sys.path.insert(0, _p)

import numpy as np
import ml_dtypes

import concourse.bacc as bacc
import concourse.bass as bass
import concourse.mybir as mybir
import concourse.tile as tile
from concourse.bass_utils import run_bass_kernel_spmd
from concourse.masks import make_identity

F32 = mybir.dt.float32
F16 = mybir.dt.float16
F8 = mybir.dt.float8e4
I32 = mybir.dt.int32
DR = mybir.MatmulPerfMode.DoubleRow

B, S, D = 4, 4096, 1024
H, DH, C = 16, 64, 64
TOK = 2048          # owned tokens per core
TH = 2176           # 64 zero-pad + 64 halo + 2048 owned
NT = TH // 128      # 17 token tiles
EPS = 1e-5
WSCALE = 16.0       # host pre-scale on the fp8 qk weights
MAGIC = 0x5F3759DF  # Quake rsqrt seed

BLOCKS = [(0, 512), (512, 512), (1024, 512), (1536, 512), (2048, 128)]


def _chunks_of_block(b):
    t0, nb = BLOCKS[b]
    return [c for c in range(32) if t0 <= 128 + 64 * c < t0 + nb]


def _out_tiles_of_block(b):
    return sorted({(c + 2) // 2 for c in _chunks_of_block(b)})


def build_nc(nblocks=5, newton=True, qk_fp8=False, use_gpsimd=False,
             paired=True, twopass=True, ydmat=False):
    nc = bacc.Bacc("TRN2", target_bir_lowering=False, debug=False)

    hid_d = nc.dram_tensor("hid", [TH, D], F16, kind="ExternalInput").ap()
    rin_d = nc.dram_tensor("rin", [TH, D], F16, kind="ExternalInput").ap()
    wqk_d = nc.dram_tensor("wqk", [D, 2 * D], F8 if qk_fp8 else F16,
                           kind="ExternalInput").ap()
    wv_d = nc.dram_tensor("wv", [D, D], F16, kind="ExternalInput").ap()
    wo_d = nc.dram_tensor("wo", [D, D], F16, kind="ExternalInput").ap()
    # masks[p, m, 64]: m=0: chunk-0 mask, m=1: band mask (one 64-query chunk;
    # broadcast across the 8 heads), multiplicative fp16 (1/64 in band).
    msk_d = nc.dram_tensor("masks", [128, 2, 64], F16, kind="ExternalInput").ap()

    out_d = nc.dram_tensor("out", [TOK, D], F16, kind="ExternalOutput").ap()
    res_d = nc.dram_tensor("res", [TOK, D], F16, kind="ExternalOutput").ap()

    hid_t = hid_d.rearrange("(t p) d -> t p d", p=128)
    rin_t = rin_d.rearrange("(t p) d -> t p d", p=128)
    out_t = out_d.rearrange("(t p) d -> t p d", p=128)
    res_p = res_d.rearrange("(t p) d -> p t d", p=128)

    from contextlib import ExitStack
    with tile.TileContext(nc) as tc, ExitStack() as ctx:
        singles = ctx.enter_context(tc.tile_pool(name="singles", bufs=1))
        io = ctx.enter_context(tc.tile_pool(name="io", bufs=3))
        resp = ctx.enter_context(tc.tile_pool(name="resp", bufs=2))
        scrp = ctx.enter_context(tc.tile_pool(name="scrp", bufs=1))
        nrm = ctx.enter_context(tc.tile_pool(name="nrm", bufs=2))
        xp = ctx.enter_context(tc.tile_pool(name="xp", bufs=2))
        xtp = ctx.enter_context(tc.tile_pool(name="xtp", bufs=2))
        x8p = ctx.enter_context(tc.tile_pool(name="x8p", bufs=2))
        qtp = ctx.enter_context(tc.tile_pool(name="qtp", bufs=2))
        ktp = ctx.enter_context(tc.tile_pool(name="ktp", bufs=2))
        vp = ctx.enter_context(tc.tile_pool(name="vp", bufs=2))
        vp1 = ctx.enter_context(tc.tile_pool(name="vp1", bufs=1))
        att = ctx.enter_context(tc.tile_pool(name="att", bufs=2))
        rcp = ctx.enter_context(tc.tile_pool(name="rcp", bufs=3))
        ybp = ctx.enter_context(tc.tile_pool(name="ybp", bufs=2))
        ytp = ctx.enter_context(tc.tile_pool(name="ytp", bufs=2))
        obp = ctx.enter_context(tc.tile_pool(name="obp", bufs=2))
        pp = ctx.enter_context(tc.tile_pool(name="pp", bufs=2, space="PSUM"))
        scp = ctx.enter_context(tc.tile_pool(name="scp", bufs=2, space="PSUM"))
        ypp = ctx.enter_context(tc.tile_pool(name="ypp", bufs=2, space="PSUM"))

        # ---- constants / weights ----
        # weight DMAs are split per k-tile and emitted inside block 0 (after
        # its input loads) so the first norm tiles stream in and start compute
        # before the 8MB of weights saturates HBM; each QK/V/O matmul only
        # depends on its own k-tile chunk.
        wqk_sb = singles.tile([128, 8, 2 * D], F8 if qk_fp8 else F16)
        wv_sb = singles.tile([128, 8, D], F16)
        wo_sb = singles.tile([128, 8, D], F16)
        msk_sb = singles.tile([128, 2, 64], F16)
        nc.sync.dma_start(msk_sb[:], msk_d)
        wqk_r = wqk_d.rearrange("(ko ki) m -> ki ko m", ki=128)
        wv_r = wv_d.rearrange("(ko ki) m -> ki ko m", ki=128)
        wo_r = wo_d.rearrange("(ko ki) m -> ki ko m", ki=128)

        def emit_weight_dmas():
            for kt in range(8):
                nc.sync.dma_start(wqk_sb[:, kt, :], wqk_r[:, kt, :])
            for kt in range(8):
                nc.sync.dma_start(wv_sb[:, kt, :], wv_r[:, kt, :])
            for kt in range(8):
                nc.sync.dma_start(wo_sb[:, kt, :], wo_r[:, kt, :])
        ident = singles.tile([128, 128], F16)
        make_identity(nc, ident[:])
        zeros_sb = singles.tile([128, 128], F16)
        nc.vector.memset(zeros_sb[:], 0.0)
        # qTz as a manually-alternated pair of singles buffers: the zero
        # half-partitions are written ONCE here (like weights), instead of
        # 2.3us of DVE broadcast-copies per block; the data halves get
        # normal WAR tracking as blocks alternate buffers.
        qTz_bufs = []
        zsrc = zeros_sb[0:64, :].rearrange("p (g s tok) -> p g s tok", g=1, s=1)
        for bi in range(2):
            qz = singles.tile([128, 8, 2, 512], F16, name=f"qTz{bi}")
            for par in range(2):
                dst = qz[64 * (1 - par):64 * (2 - par), :, par, :]
                nc.vector.tensor_copy(
                    dst.rearrange("p g (s tok) -> p g s tok", tok=128),
                    zsrc.to_broadcast([64, 8, 4, 128]))
            qTz_bufs.append(qz)

        kT_prev = None
        v_prev = None
        pend_out = None
        pend_dma = None

        for b, (t0, nb) in enumerate(BLOCKS[:nblocks]):
            ntile = nb // 128
            xT_b = xtp.tile([128, 8, 512], F16, tag="xT")
            x8_b = x8p.tile([128, 8, 512], F8, tag="x8") if qk_fp8 else None
            res_b = resp.tile([128, 4, D], F16, tag="res")
            ms_b = nrm.tile([128, 4], F32, tag="ms")

            # ---- per-tile norm: add, stats, rsqrt, scale, transpose ----
            # fully per-tile so the first tile's transposes (and the split QK
            # sub-matmuls that depend on them) start ~3 tiles earlier than a
            # batched-rsqrt formulation would allow
            def emit_rsqrt(ms_ap, i):
                """inv = rsqrt(ms[:, i]) on the DVE (Newton, bit-trick seed)."""
                nc.vector.tensor_scalar_add(ms_ap[:, i:i + 1], ms_ap[:, i:i + 1],
                                            EPS)
                if newton:
                    msi = ms_ap.bitcast(I32)
                    sh = nrm.tile([128, 1], I32, tag="sh")
                    nc.vector.tensor_scalar(sh[:], msi[:, i:i + 1], 1, None,
                                            mybir.AluOpType.logical_shift_right)
                    zi = nrm.tile([128, 1], I32, tag="zi")
                    nc.vector.tensor_scalar(zi[:], sh[:], -1, MAGIC,
                                            mybir.AluOpType.mult,
                                            mybir.AluOpType.add)
                    z = zi.bitcast(F32)
                    u = nrm.tile([128, 1], F32, tag="u")
                    for _ in range(2):
                        nc.vector.tensor_mul(u[:], z[:], z[:])
                        nc.vector.tensor_mul(u[:], u[:], ms_ap[:, i:i + 1])
                        nc.vector.tensor_scalar(u[:], u[:], -0.5, 1.5,
                                                mybir.AluOpType.mult,
                                                mybir.AluOpType.add)
                        nc.vector.tensor_mul(z[:], z[:], u[:])
                    return z
                sq = nrm.tile([128, 1], F32, tag="sq")
                nc.scalar.activation(sq[:], ms_ap[:, i:i + 1],
                                     mybir.ActivationFunctionType.Sqrt)
                z = nrm.tile([128, 1], F32, tag="z")
                nc.vector.reciprocal(z[:], sq[:])
                return z

            for i in range(ntile):
                t = t0 // 128 + i
                ht = io.tile([128, D], F16, tag="hid")
                nc.sync.dma_start(ht[:], hid_t[t])
                rt = io.tile([128, D], F16, tag="rin")
                nc.sync.dma_start(rt[:], rin_t[t])
                nc.vector.tensor_add(res_b[:, i, :], ht[:], rt[:])
                scr = scrp.tile([128, D], F16, tag="scr")
                nc.scalar.activation(scr[:], res_b[:, i, :],
                                     mybir.ActivationFunctionType.Square,
                                     scale=float(1.0 / np.sqrt(D)),
                                     accum_out=ms_b[:, i:i + 1])
                z = emit_rsqrt(ms_b, i)
                x16 = xp.tile([128, D], F16, tag="x16")
                nc.vector.tensor_scalar_mul(x16[:], res_b[:, i, :], z[:])
                for g in range(2):
                    ps = pp.tile([128, 4, 128], F16, tag="mm", name="trq")
                    for k4 in range(4):
                        kt = g * 4 + k4
                        nc.tensor.transpose(ps[:, k4, :],
                                            x16[:, kt * 128:(kt + 1) * 128],
                                            ident[:])
                    nc.vector.tensor_copy(
                        xT_b[:, g * 4:(g + 1) * 4, i * 128:(i + 1) * 128], ps[:])
                if qk_fp8:
                    eng = nc.gpsimd if use_gpsimd else nc.vector
                    eng.tensor_copy(x8_b[:, :, i * 128:(i + 1) * 128],
                                    xT_b[:, :, i * 128:(i + 1) * 128])

            if b == 0:
                emit_weight_dmas()
            # res writeback on the SP queue right after the adds complete
            tlo = 1 if b == 0 else 0
            nc.sync.dma_start(
                res_p[:, t0 // 128 + tlo - 1:t0 // 128 + ntile - 1, :],
                res_b[:, tlo:ntile, :])

            # ---- q/k projection (fp8 DoubleRow, feature-major out) ----
            # qTz[p, hp, parity, tok]: head h = 2*hp + parity; its features sit
            # at partitions (h%2)*64..+64, the other half is zero.
            qTz = qTz_bufs[b % 2]
            kT_b = ktp.tile([128, 8, 576], F16, tag="kT")
            if b > 0:
                nc.vector.tensor_copy(kT_b[:, :, 0:64], kT_prev[:, :, 512:576])
            wsc = WSCALE if qk_fp8 else 1.0
            for mt in range(16):
                ps = pp.tile([128, 512], F32, tag="mm")
                if qk_fp8:
                    for u in range(4):
                        nc.tensor.matmul(
                            ps[:, :nb],
                            wqk_sb[:, 2 * u:2 * u + 2, mt * 128:(mt + 1) * 128],
                            x8_b[:, 2 * u:2 * u + 2, :nb],
                            start=(u == 0), stop=(u == 3),
                            perf_mode=DR)
                else:
                    # split the moving operand per 128-token tile: the first
                    # sub-matmuls only depend on the first tile's transposes,
                    # so the PE starts each block's projections ~3 norm-tiles
                    # earlier (kills the startup and block-boundary PE gaps).
                    # block 0 tile 0 is pad+halo whose queries are never read
                    # (out tiles start at chunk 0 = rows 128+), so skip its q.
                    i0 = 1 if (b == 0 and mt < 8) else 0
                    for i in range(i0, ntile):
                        for kt in range(8):
                            nc.tensor.matmul(
                                ps[:, i * 128:(i + 1) * 128],
                                wqk_sb[:, kt, mt * 128:(mt + 1) * 128],
                                xT_b[:, kt, i * 128:(i + 1) * 128],
                                start=(kt == 0), stop=(kt == 7))
                if mt < 8:
                    q0 = 128 if b == 0 else 0
                    nc.scalar.activation(qTz[0:64, mt, 0, q0:nb],
                                         ps[0:64, q0:nb],
                                         mybir.ActivationFunctionType.Copy,
                                         scale=1.0 / (wsc * 8.0))
                    nc.scalar.activation(qTz[64:128, mt, 1, q0:nb],
                                         ps[64:128, q0:nb],
                                         mybir.ActivationFunctionType.Copy,
                                         scale=1.0 / (wsc * 8.0))
                else:
                    nc.scalar.activation(kT_b[:, mt - 8, 64:64 + nb], ps[:, :nb],
                                         mybir.ActivationFunctionType.Copy,
                                         scale=1.0 / wsc)

            # ---- v projection (fp16, token-major, head-interleaved + ones) ----
            v_b = vp.tile([128, 5, 16 * 65], F16, tag="v")
            ones_view = v_b[:, 1:5, :].rearrange("p s (h e) -> p s h e", e=65)
            nc.vector.memset(ones_view[:, :, :, 64:65], 1.0)
            if b > 0:
                nc.vector.tensor_copy(v_b[:, 0, :], v_prev[:, 4, :])
            for i in range(ntile):
                vslot = v_b[:, i + 1, :].rearrange("p (h e) -> p h e", e=65)
                for nh in range(2):
                    ps = pp.tile([128, 512], F32, tag="mm")
                    for kt in range(8):
                        nc.tensor.matmul(ps[:],
                                         xT_b[:, kt, i * 128:(i + 1) * 128],
                                         wv_sb[:, kt, nh * 512:(nh + 1) * 512],
                                         start=(kt == 0), stop=(kt == 7))
                    nc.scalar.activation(
                        vslot[:, nh * 8:(nh + 1) * 8, 0:64],
                        ps[:].rearrange("p (h e) -> p h e", e=64),
                        mybir.ActivationFunctionType.Copy)

            # ---- phase-1 v tiles (for even chunks) via SBUF->SBUF DMA (SP) ----
            cs = _chunks_of_block(b)
            ms_needed = sorted({c // 2 for c in cs if c % 2 == 0})
            v1_b = vp1.tile([128, 4, 16 * 65], F16, tag="v1")
            for m in ms_needed:
                s = m - (4 * b - 1)
                s0 = m - 4 * b + 1      # v_b slot holding global tile m
                nc.sync.dma_start(v1_b[0:64, s, :], v_b[64:128, s0, :])
                nc.sync.dma_start(v1_b[64:128, s, :], v_b[0:64, s0 + 1, :])

            # ---- attention per 128-token out-tile, software-pipelined:
            # scores/exp/mask/PV for both head-banks first, then the
            # normalizes, and the (PE-heavy) output projection of the
            # PREVIOUS out-tile so PE never waits on the DVE normalize.
            def emit_out(t_prev, yT_prev):
                nonlocal pend_dma
                osb = obp.tile([128, D], F16, tag="osb")
                for nh in range(2):
                    ps = pp.tile([128, 512], F32, tag="mm", name="po")
                    for kt in range(8):
                        nc.tensor.matmul(ps[:], yT_prev[:, kt, :],
                                         wo_sb[:, kt, nh * 512:(nh + 1) * 512],
                                         start=(kt == 0), stop=(kt == 7))
                    nc.scalar.activation(osb[:, nh * 512:(nh + 1) * 512], ps[:],
                                         mybir.ActivationFunctionType.Copy)
                # out writeback deferred one more stage so the Act-queue DMA
                # never waits on an in-flight output projection
                if pend_dma is not None:
                    nc.scalar.dma_start(out_t[pend_dma[0] - 1], pend_dma[1][:])
                pend_dma = (t_prev, osb)

            for t in _out_tiles_of_block(b):
                # yT transpose of the previous out-tile: issued now so the
                # XBAR DMA completes while this tile's attention runs
                pend_tr = None
                if pend_out is not None:
                    yT_t = ytp.tile([128, 8, 128], F16, tag="yT")
                    if ydmat:
                        nc.scalar.dma_start_transpose(yT_t[:], pend_out[1][:])
                    else:
                        for g in range(2):
                            ps = pp.tile([128, 4, 128], F16, tag="mm", name="try")
                            for k4 in range(4):
                                kt = g * 4 + k4
                                nc.tensor.transpose(
                                    ps[:, k4, :],
                                    pend_out[1][:, kt * 128:(kt + 1) * 128],
                                    ident[:])
                            nc.vector.tensor_copy(yT_t[:, g * 4:(g + 1) * 4, :],
                                                  ps[:])
                    pend_tr = (pend_out[0], yT_t)
                yblk = ybp.tile([128, D], F16, tag="yblk")
                y_pss = []
                for hb in range(2):
                    y_ps = ypp.tile([128, 2, 512], F32, tag="y")
                    y_pss.append(y_ps)
                    # pass 1: both 64-query chunks' scores + exp + mask;
                    # pass 2: their PV matmuls. PE streams the second chunk's
                    # scores while Act/DVE run the first chunk's exp/mask, so
                    # the PV never stalls the Tensor engine.
                    expSs = []
                    for which in range(2 if twopass else 0):
                        c = 2 * t - 2 + which
                        ko = 128 + 64 * c - t0
                        qo = 128 + 64 * c - t0
                        sc_ps = scp.tile([128, 512], F32, tag="sc")
                        if paired:
                            for u in range(4):
                                nc.tensor.matmul(
                                    sc_ps[:, u * 128:(u + 1) * 128],
                                    kT_b[:, 4 * hb + u, ko:ko + 128],
                                    qTz[:, 4 * hb + u, :, qo:qo + 64],
                                    start=True, stop=True)
                        else:
                            for h2 in range(8):
                                nc.tensor.matmul(
                                    sc_ps[:, h2 * 64:(h2 + 1) * 64],
                                    kT_b[:, (hb * 8 + h2) // 2, ko:ko + 128],
                                    qTz[:, (hb * 8 + h2) // 2, h2 % 2,
                                        qo:qo + 64],
                                    start=True, stop=True)
                        expS = att.tile([128, 512], F16, tag="expS")
                        nc.scalar.activation(expS[:], sc_ps[:],
                                             mybir.ActivationFunctionType.Exp)
                        mi = 0 if c == 0 else 1
                        mskb = msk_sb[:, mi:mi + 1, :].to_broadcast([128, 8, 64])
                        expS_h = expS.rearrange("p (h q) -> p h q", q=64)
                        nc.vector.tensor_mul(expS_h, expS_h, mskb)
                        expSs.append(expS)
                    for which in range(2):
                        c = 2 * t - 2 + which
                        if twopass:
                            expS = expSs[which]
                        else:
                            ko = 128 + 64 * c - t0
                            qo = 128 + 64 * c - t0
                            sc_ps = scp.tile([128, 512], F32, tag="sc")
                            for u in range(4):
                                nc.tensor.matmul(
                                    sc_ps[:, u * 128:(u + 1) * 128],
                                    kT_b[:, 4 * hb + u, ko:ko + 128],
                                    qTz[:, 4 * hb + u, :, qo:qo + 64],
                                    start=True, stop=True)
                            expS = att.tile([128, 512], F16, tag="expS")
                            nc.scalar.activation(expS[:], sc_ps[:],
                                                 mybir.ActivationFunctionType.Exp)
                            mi = 0 if c == 0 else 1
                            mskb = msk_sb[:, mi:mi + 1, :].to_broadcast([128, 8, 64])
                            expS_h = expS.rearrange("p (h q) -> p h q", q=64)
                            nc.vector.tensor_mul(expS_h, expS_h, mskb)
                        # PV: contract over the 128-key window
                        if c % 2 == 0:
                            vt = v1_b[:, c // 2 - (4 * b - 1), :]
                        else:
                            vt = v_b[:, (c + 1) // 2 - 4 * b + 1, :]
                        vtile = vt.rearrange("p (h e) -> p h e", e=65)
                        for h2 in range(8):
                            h = hb * 8 + h2
                            oap = y_ps[which * 64:(which + 1) * 64, h2 // 4,
                                       (h2 % 4) * 65:(h2 % 4) * 65 + 65]
                            nc.tensor.matmul(
                                oap, expS[:, h2 * 64:(h2 + 1) * 64],
                                vtile[:, h, :],
                                start=True, stop=True,
                                tile_position=(0, which * 64))
                for hb in range(2):
                    # normalize: per-query reciprocal of denominator column
                    ybank = y_pss[hb][:, :, 0:260].rearrange(
                        "p b (h e) -> p b h e", e=65)
                    rc = rcp.tile([128, 2, 4, 1], F32, tag="rc")
                    nc.vector.reciprocal(rc[:], ybank[:, :, :, 64:65])
                    ydst = yblk[:, hb * 512:(hb + 1) * 512].rearrange(
                        "p (b h e) -> p b h e", b=2, h=4)
                    nc.vector.tensor_mul(ydst, ybank[:, :, :, 0:64],
                                         rc[:].to_broadcast([128, 2, 4, 64]))
                if pend_tr is not None:
                    emit_out(*pend_tr)
                pend_out = (t, yblk)

            kT_prev = kT_b
            v_prev = v_b

        if pend_out is not None:
            yT_t = ytp.tile([128, 8, 128], F16, tag="yT")
            if ydmat:
                nc.scalar.dma_start_transpose(yT_t[:], pend_out[1][:])
            else:
                for g in range(2):
                    ps = pp.tile([128, 4, 128], F16, tag="mm", name="try")
                    for k4 in range(4):
                        kt = g * 4 + k4
                        nc.tensor.transpose(
                            ps[:, k4, :],
                            pend_out[1][:, kt * 128:(kt + 1) * 128], ident[:])
                    nc.vector.tensor_copy(yT_t[:, g * 4:(g + 1) * 4, :], ps[:])
            emit_out(pend_out[0], yT_t)
        if pend_dma is not None:
            nc.scalar.dma_start(out_t[pend_dma[0] - 1], pend_dma[1][:])

    nc.compile()
    return nc


def _build_masks(seq_start: bool) -> np.ndarray:
    j = np.arange(128)[:, None]   # key pos in window
    i = np.arange(64)[None, :]    # query pos in chunk
    band = (j >= i) & (j <= i + 64)
    m0 = band & (j >= 64)         # chunk 0 at sequence start

    def vals(m):
        return (m.astype(np.float32) / 64.0).astype(np.float16)

    out = np.empty((128, 2, 64), np.float16)
    out[:, 0, :] = vals(m0 if seq_start else band)
    out[:, 1, :] = vals(band)
    return out


_NC = None
_NC_KEY = None


def kernel(hidden_states, residual, norm_weight, w_qkv, w_out, trace=False,
           **flags):
    global _NC, _NC_KEY
    key = tuple(sorted(flags.items()))
    if _NC is None or _NC_KEY != key:
        _NC = build_nc(**flags)
        _NC_KEY = key
    nc = _NC

    hidden_states = np.asarray(hidden_states, np.float32)
    residual = np.asarray(residual, np.float32)
    norm_weight = np.asarray(norm_weight, np.float32)
    w_qkv = np.asarray(w_qkv, np.float32)
    w_out = np.asarray(w_out, np.float32)

    if flags.get("qk_fp8", False):
        wqk_h = (norm_weight[:, None] * w_qkv[:, :2 * D] * WSCALE).astype(
            ml_dtypes.float8_e4m3)
    else:
        wqk_h = (norm_weight[:, None] * w_qkv[:, :2 * D]).astype(np.float16)
    wv16 = (norm_weight[:, None] * w_qkv[:, 2 * D:]).astype(np.float16)
    wo16 = w_out.astype(np.float16)

    hid16 = hidden_states.astype(np.float16)
    rin16 = residual.astype(np.float16)

    in_maps = []
    for core in range(8):
        b, s = core // 2, core % 2
        hid = np.zeros((TH, D), np.float16)
        rin = np.zeros((TH, D), np.float16)
        if s == 1:
            hid[64:128] = hid16[b, TOK - 64:TOK]
            rin[64:128] = rin16[b, TOK - 64:TOK]
        hid[128:] = hid16[b, s * TOK:(s + 1) * TOK]
        rin[128:] = rin16[b, s * TOK:(s + 1) * TOK]
        in_maps.append({
            "hid": hid, "rin": rin,
            "wqk": wqk_h, "wv": wv16, "wo": wo16,
            "masks": _build_masks(seq_start=(s == 0)),
        })

    r = run_bass_kernel_spmd(nc, in_maps, list(range(8)), trace=trace)
    if trace:
        kernel.last_exec_ns = r.exec_time_ns
        kernel.last_results = r
    kernel.last_in_maps = in_maps

    out = np.empty((B, S, D), np.float32)
    res = np.empty((B, S, D), np.float32)
    for core in range(8):
        b, s = core // 2, core % 2
        out[b, s * TOK:(s + 1) * TOK] = r.results[core]["out"].astype(np.float32)
        res[b, s * TOK:(s + 1) * TOK] = r.results[core]["res"].astype(np.float32)
    return out, res


def bench_chain(in_maps, reps=16, iters=6):
    """True on-device per-exec time: chain `reps` NEFF executions in one
    jitted call (each feeding the previous outputs into the donated output
    slots, so XLA can't CSE or reorder them), then difference against a
    single-exec call. Removes host/axon dispatch overhead from the metric."""
    import time

    import jax
    from jax.experimental.shard_map import shard_map
    from jax.sharding import Mesh, NamedSharding, PartitionSpec

    from concourse import bass2jax, mybir as _mb

    nc = _NC
    bass2jax.install_neuronx_cc_hook()
    partition_name = nc.partition_id_tensor.name if nc.partition_id_tensor else None

    in_names, out_names, out_avals = [], [], []
    for alloc in nc.m.functions[0].allocations:
        if not isinstance(alloc, _mb.MemoryLocationSet):
            continue
        name = alloc.memorylocations[0].name
        if alloc.kind == "ExternalInput":
            if name != partition_name:
                in_names.append(name)
        elif alloc.kind == "ExternalOutput":
            out_names.append(name)
            out_avals.append(jax.core.ShapedArray(
                tuple(alloc.tensor_shape), _mb.dt.np(alloc.dtype)))
    n_params = len(in_names)
    all_in = list(in_names) + list(out_names)
    if partition_name is not None:
        all_in.append(partition_name)

    def _exec_once(params, z):
        operands = list(params) + list(z)
        if partition_name is not None:
            operands.append(bass2jax.partition_id_tensor())
        return bass2jax._bass_exec_p.bind(
            *operands,
            out_avals=tuple(out_avals),
            in_names=tuple(all_in),
            out_names=tuple(out_names),
            lowering_input_output_aliases=(),
            sim_require_finite=True,
            sim_require_nnan=True,
            nc=nc,
        )

    def _body_n(n):
        def f(*args):
            params = args[:n_params]
            z = list(args[n_params:])
            for _ in range(n):
                z = list(_exec_once(params, z))
            return tuple(z)
        return f

    devices = jax.devices()[:8]
    mesh = Mesh(np.asarray(devices), ("core",))
    nio = n_params + len(out_names)
    concat_in = [np.concatenate([np.asarray(in_maps[c][n]) for c in range(8)],
                                axis=0) for n in in_names]
    shd = NamedSharding(mesh, PartitionSpec("core"))
    dev_in = [jax.device_put(a, shd) for a in concat_in]
    zeros_np = [np.zeros((8 * a.shape[0], *a.shape[1:]), a.dtype)
                for a in out_avals]

    results = {}
    for n in (1, reps):
        fn = jax.jit(
            shard_map(_body_n(n), mesh=mesh,
                      in_specs=(PartitionSpec("core"),) * nio,
                      out_specs=(PartitionSpec("core"),) * len(out_names),
                      check_rep=False),
            keep_unused=True)
        best = None
        for _ in range(iters):
            dz = [jax.device_put(z, shd) for z in zeros_np]
            jax.block_until_ready(dz)
            t0 = time.perf_counter()
            outs = fn(*dev_in, *dz)
            jax.block_until_ready(outs)
            dt = time.perf_counter() - t0
            best = dt if best is None else min(best, dt)
        results[n] = best
    per_exec = (results[reps] - results[1]) / (reps - 1)
    return per_exec, results


def bench(in_maps, iters=20):
    """Steady-state wall time per execution of the compiled NEFF across the
    8 cores (includes PJRT/axon dispatch overhead; upper bound on HW time)."""
    import time

    import jax
    from jax.experimental.shard_map import shard_map
    from jax.sharding import Mesh, NamedSharding, PartitionSpec

    from concourse import bass2jax, mybir as _mb

    nc = _NC
    bass2jax.install_neuronx_cc_hook()
    partition_name = nc.partition_id_tensor.name if nc.partition_id_tensor else None

    in_names, out_names, out_avals, zero_outs = [], [], [], []
    for alloc in nc.m.functions[0].allocations:
        if not isinstance(alloc, _mb.MemoryLocationSet):
            continue
        name = alloc.memorylocations[0].name
        if alloc.kind == "ExternalInput":
            if name != partition_name:
                in_names.append(name)
        elif alloc.kind == "ExternalOutput":
            shape = tuple(alloc.tensor_shape)
            dtype = _mb.dt.np(alloc.dtype)
            out_names.append(name)
            out_avals.append(jax.core.ShapedArray(shape, dtype))
            zero_outs.append(np.zeros(shape, dtype))
    n_params = len(in_names)
    n_outs = len(out_avals)
    all_in = list(in_names) + list(out_names)
    if partition_name is not None:
        all_in.append(partition_name)
    donate = tuple(range(n_params, n_params + n_outs))

    def _body(*args):
        operands = list(args)
        if partition_name is not None:
            operands.append(bass2jax.partition_id_tensor())
        return tuple(bass2jax._bass_exec_p.bind(
            *operands,
            out_avals=tuple(out_avals),
            in_names=tuple(all_in),
            out_names=tuple(out_names),
            lowering_input_output_aliases=(),
            sim_require_finite=True,
            sim_require_nnan=True,
            nc=nc,
        ))

    devices = jax.devices()[:8]
    mesh = Mesh(np.asarray(devices), ("core",))
    in_specs = (PartitionSpec("core"),) * (n_params + n_outs)
    out_specs = (PartitionSpec("core"),) * n_outs
    sharded = jax.jit(
        shard_map(_body, mesh=mesh, in_specs=in_specs, out_specs=out_specs,
                  check_rep=False),
        donate_argnums=donate, keep_unused=True)

    concat_in = [np.concatenate([np.asarray(in_maps[c][n]) for c in range(8)], axis=0)
                 for n in in_names]
    shd = NamedSharding(mesh, PartitionSpec("core"))
    dev_in = [jax.device_put(a, shd) for a in concat_in]
    zeros_np = [np.zeros((8 * z.shape[0], *z.shape[1:]), z.dtype) for z in zero_outs]

    times = []
    outs = None
    for it in range(iters):
        dz = [jax.device_put(z, shd) for z in zeros_np]
        jax.block_until_ready(dz)
        t0 = time.perf_counter()
        outs = sharded(*dev_in, *dz)
        jax.block_until_ready(outs)
        times.append(time.perf_counter() - t0)
    return times, outs
